# revision 1
# baseline (speedup 1.0000x reference)
"""LocalLoraAttention Trainium2 kernel: 8-core head-sharded, LoRA folded into weights.

Sharding: core c owns heads 2c,2c+1 (256 dims). LoRA is folded on host:
W_d = W + 2*B_d@A_d, W_v = W + 2*B_v@A_v; per-token modal mix becomes
out = (x*m_d)@W_d^T + (x*m_v)@W_v^T (masks pre-applied to x on host for qkv,
applied on device to attention output for the o projection). Each core
computes its 2 heads' q/k/v (transposed layout), RoPE, causal attention
(scores^T orientation, exp without max-subtraction, ones-matmul denominator),
and a full-width partial o-projection; host sums the 8 partials.
"""
import sys
sys.path.insert(0, '/opt/trn_rl_repo')
import numpy as np
import ml_dtypes

import concourse.bass as bass
import concourse.tile as tile
import concourse.mybir as mybir
from concourse import bass_utils

B, S, H, NH, HD, R = 2, 2048, 2048, 16, 128, 128
LORA_SCALE = 2.0
NCORES = 8
DPC = H // NCORES          # 256 out-dims per core (2 heads)
TOK = B * S                # 4096
NB = 256                   # phase A token block
QB = 512                   # attention q block
NCH = H // 128             # 16 contraction chunks
NKT = S // 128             # 16 k-tiles per batch
NQB = S // QB              # 4 q blocks per batch
F32 = mybir.dt.float32
BF16 = mybir.dt.bfloat16
ISQ = float(1.0 / np.sqrt(HD))

_CACHE = {}


def _split_waits(nc, max_waits=1):
    """This walrus build allows only one sync-wait per instruction; split
    extras onto preceding NOPs on the same engine."""
    ctr = 0
    for fn in nc.m.functions:
        for bb in fn.blocks:
            out = []
            for inst in bb.instructions:
                si = getattr(inst, 'sync_info', None)
                waits = list(si.on_wait) if si and si.on_wait else []
                if len(waits) > max_waits:
                    chunks = [waits[i:i + max_waits]
                              for i in range(0, len(waits), max_waits)]
                    for ch in chunks[:-1]:
                        ctr += 1
                        nop = mybir.InstNoOp(
                            name=f"Wsplit-{ctr}", ins=[], outs=[],
                            sync_info=mybir.SyncInfo(on_wait=ch, on_update=[]))
                        nop.engine = inst.engine
                        out.append(nop)
                    si.on_wait = chunks[-1]
                out.append(inst)
            bb.instructions[:] = out


def _build():
    import concourse.tile_utils as tile_utils
    tile_utils.max_sbuf_usage = 204 * 1024

    nc = bass.Bass("TRN2", target_bir_lowering=False)
    xd = nc.dram_tensor("xd", [H, TOK], BF16, kind="ExternalInput")
    xv = nc.dram_tensor("xv", [H, TOK], BF16, kind="ExternalInput")
    wq_d = nc.dram_tensor("wq_d", [H, DPC], BF16, kind="ExternalInput")
    wq_v = nc.dram_tensor("wq_v", [H, DPC], BF16, kind="ExternalInput")
    wk_d = nc.dram_tensor("wk_d", [H, DPC], BF16, kind="ExternalInput")
    wk_v = nc.dram_tensor("wk_v", [H, DPC], BF16, kind="ExternalInput")
    wv_d = nc.dram_tensor("wv_d", [H, DPC], BF16, kind="ExternalInput")
    wv_v = nc.dram_tensor("wv_v", [H, DPC], BF16, kind="ExternalInput")
    wo_d = nc.dram_tensor("wo_d", [DPC, H], BF16, kind="ExternalInput")
    wo_v = nc.dram_tensor("wo_v", [DPC, H], BF16, kind="ExternalInput")
    mdb = nc.dram_tensor("mdb", [128, TOK], F32, kind="ExternalInput")
    mvb = nc.dram_tensor("mvb", [128, TOK], F32, kind="ExternalInput")
    cosT = nc.dram_tensor("cosT", [128, S], F32, kind="ExternalInput")
    sinTs = nc.dram_tensor("sinTs", [128, S], F32, kind="ExternalInput")
    cmt = nc.dram_tensor("cmt", [128, 4 * QB], F32, kind="ExternalInput")
    outp = nc.dram_tensor("outp", [H, TOK], F32, kind="ExternalOutput")

    with tile.TileContext(nc) as tc:
        with tc.tile_pool(name="wp", bufs=1) as wp, \
             tc.tile_pool(name="qkv", bufs=1) as qkvp, \
             tc.tile_pool(name="xs", bufs=2) as xs, \
             tc.tile_pool(name="rw", bufs=3) as rw, \
             tc.tile_pool(name="ew", bufs=1) as ew, \
             tc.tile_pool(name="at", bufs=2) as atp, \
             tc.tile_pool(name="ad", bufs=2) as adp, \
             tc.tile_pool(name="osp", bufs=2) as osp, \
             tc.tile_pool(name="ps", bufs=8, space="PSUM") as psp:

            def w3d(dram):  # [H, DPC] -> sbuf [128, NCH, DPC]
                t = wp.tile([128, NCH, DPC], BF16, tag=dram.name)
                nc.sync.dma_start(
                    out=t, in_=dram.rearrange("(c p) d -> p c d", p=128))
                return t

            wq = {'d': w3d(wq_d), 'v': w3d(wq_v)}
            wk = {'d': w3d(wk_d), 'v': w3d(wk_v)}
            wv = {'d': w3d(wv_d), 'v': w3d(wv_v)}
            wo = {}
            for nm, dram in (('d', wo_d), ('v', wo_v)):
                t = wp.tile([128, 2, H], BF16, tag='wo' + nm)
                nc.sync.dma_start(
                    out=t, in_=dram.rearrange("(c p) o -> p c o", p=128))
                wo[nm] = t
            cos_sb = wp.tile([128, S], F32, tag='cos')
            nc.sync.dma_start(out=cos_sb, in_=cosT[:, :])
            sin_sb = wp.tile([128, S], F32, tag='sin')
            nc.sync.dma_start(out=sin_sb, in_=sinTs[:, :])
            cm_sb = wp.tile([128, 4, QB], F32, tag='cm')
            nc.sync.dma_start(
                out=cm_sb, in_=cmt.rearrange("p (j q) -> p j q", j=4))
            ones128 = wp.tile([128, 1], BF16, tag='o128')
            nc.vector.memset(ones128, 1.0)
            ones1 = wp.tile([1, 128], F32, tag='o1')
            nc.vector.memset(ones1, 1.0)

            qT = qkvp.tile([128, 2, TOK], BF16, tag='qT')
            kT = qkvp.tile([128, 2, TOK], BF16, tag='kT')
            v_sb = qkvp.tile([128, B * NKT, 256], BF16, tag='v')

            xd3 = xd.rearrange("(c p) t -> p c t", p=128)
            xv3 = xv.rearrange("(c p) t -> p c t", p=128)

            for b in range(B):
                # ---- phase A: qkv projections for batch b ----
                for t in range(S // NB):
                    tok0 = b * S + t * NB
                    s0 = t * NB
                    xdt = xs.tile([128, NCH, NB], BF16, tag='xd')
                    nc.sync.dma_start(out=xdt, in_=xd3[:, :, tok0:tok0 + NB])
                    xvt = xs.tile([128, NCH, NB], BF16, tag='xv')
                    nc.sync.dma_start(out=xvt, in_=xv3[:, :, tok0:tok0 + NB])

                    for wdict, dstT in ((wq, qT), (wk, kT)):
                        for hb in range(2):
                            ps = psp.tile([128, NB], F32, tag='ps')
                            i = 0
                            for var, xt in (('d', xdt), ('v', xvt)):
                                for c in range(NCH):
                                    nc.tensor.matmul(
                                        ps,
                                        lhsT=wdict[var][:, c, hb * 128:(hb + 1) * 128],
                                        rhs=xt[:, c, :],
                                        start=(i == 0), stop=(i == 31))
                                    i += 1
                            # RoPE + cast eviction
                            scp = rw.tile([128, NB], F32, tag='scp')
                            nc.vector.tensor_copy(scp, ps)
                            sh = rw.tile([128, NB], F32, tag='sh')
                            nc.sync.dma_start(out=sh[0:64, :], in_=scp[64:128, :])
                            nc.sync.dma_start(out=sh[64:128, :], in_=scp[0:64, :])
                            r1 = rw.tile([128, NB], F32, tag='r1')
                            nc.vector.tensor_mul(r1, ps, cos_sb[:, s0:s0 + NB])
                            r2 = rw.tile([128, NB], F32, tag='r2')
                            nc.vector.tensor_mul(r2, sh, sin_sb[:, s0:s0 + NB])
                            nc.vector.tensor_add(
                                dstT[:, hb, tok0:tok0 + NB], r1, r2)
                    for tt2 in range(NB // 128):
                        psv = psp.tile([128, 256], F32, tag='ps')
                        i = 0
                        for var, xt in (('d', xdt), ('v', xvt)):
                            for c in range(NCH):
                                nc.tensor.matmul(
                                    psv,
                                    lhsT=xt[:, c, tt2 * 128:(tt2 + 1) * 128],
                                    rhs=wv[var][:, c, :],
                                    start=(i == 0), stop=(i == 31))
                                i += 1
                        nc.vector.tensor_copy(
                            v_sb[:, b * NKT + (t * NB) // 128 + tt2, :], psv)

                # ---- phase B+C per q-block ----
                for qb in range(NQB):
                    q0 = b * S + qb * QB
                    attn = {}
                    for h in range(2):
                        ps_av = psp.tile([128, QB], F32, tag='ps')
                        ps_den = psp.tile([1, QB], F32, tag='ps')
                        nk = 4 * qb + 4
                        for ki in range(nk):
                            ps_s = psp.tile([128, QB], F32, tag='ps')
                            nc.tensor.matmul(
                                ps_s,
                                lhsT=kT[:, h, b * S + ki * 128: b * S + (ki + 1) * 128],
                                rhs=qT[:, h, q0:q0 + QB],
                                start=True, stop=True)
                            at = atp.tile([128, QB], BF16, tag='at')
                            j = ki - 4 * qb
                            if j >= 0:
                                e32 = ew.tile([128, QB], F32, tag='e32')
                                nc.scalar.activation(
                                    e32, ps_s,
                                    mybir.ActivationFunctionType.Exp, scale=ISQ)
                                nc.vector.tensor_mul(at, e32, cm_sb[:, j, :])
                            else:
                                nc.scalar.activation(
                                    at, ps_s,
                                    mybir.ActivationFunctionType.Exp, scale=ISQ)
                            nc.tensor.matmul(
                                ps_av,
                                lhsT=v_sb[:, b * NKT + ki, h * 128:(h + 1) * 128],
                                rhs=at, start=(ki == 0), stop=(ki == nk - 1))
                            nc.tensor.matmul(
                                ps_den, lhsT=ones128, rhs=at,
                                start=(ki == 0), stop=(ki == nk - 1))
                        rden = ew.tile([1, QB], F32, tag='rden')
                        nc.vector.reciprocal(rden, ps_den)
                        ps_b = psp.tile([128, QB], F32, tag='ps')
                        nc.tensor.matmul(ps_b, lhsT=ones1, rhs=rden,
                                         start=True, stop=True)
                        rb = ew.tile([128, QB], F32, tag='rb')
                        nc.vector.tensor_copy(rb, ps_b)
                        t1 = ew.tile([128, QB], F32, tag='t1')
                        nc.vector.tensor_mul(t1, ps_av, rb)
                        mdq = ew.tile([128, QB], F32, tag='mdq')
                        nc.sync.dma_start(out=mdq, in_=mdb[:, q0:q0 + QB])
                        mvq = ew.tile([128, QB], F32, tag='mvq')
                        nc.sync.dma_start(out=mvq, in_=mvb[:, q0:q0 + QB])
                        ad = adp.tile([128, QB], BF16, tag=f'ad{h}')
                        nc.vector.tensor_mul(ad, t1, mdq)
                        av = adp.tile([128, QB], BF16, tag=f'av{h}')
                        nc.vector.tensor_mul(av, t1, mvq)
                        attn[(h, 'd')] = ad
                        attn[(h, 'v')] = av
                    # phase C: partial o-projection for these 512 tokens
                    for ob in range(NCH):
                        ps_o = psp.tile([128, QB], F32, tag='ps')
                        i = 0
                        for var in ('d', 'v'):
                            for hl in range(2):
                                nc.tensor.matmul(
                                    ps_o,
                                    lhsT=wo[var][:, hl, ob * 128:(ob + 1) * 128],
                                    rhs=attn[(hl, var)],
                                    start=(i == 0), stop=(i == 3))
                                i += 1
                        osb = osp.tile([128, QB], F32, tag='osb')
                        nc.vector.tensor_copy(osb, ps_o)
                        nc.sync.dma_start(
                            out=outp[ob * 128:(ob + 1) * 128, q0:q0 + QB],
                            in_=osb)
    _split_waits(nc)
    return nc


def _prep(inputs):
    x = np.asarray(inputs['hidden_states'], np.float32)
    m_d = np.asarray(inputs['mask_default'], np.float32)
    m_v = np.asarray(inputs['mask_vision'], np.float32)

    def fold(Wn, An, Bn):
        W = np.asarray(inputs[Wn], np.float32)
        A = np.asarray(inputs[An], np.float32)
        Bm = np.asarray(inputs[Bn], np.float32)
        return (W + LORA_SCALE * (Bm @ A)).astype(np.float32)

    Wf = {}
    for p in 'qkvo':
        for ad in 'dv':
            Wf[(p, ad)] = fold(f'W{p}', f'{p}A_{ad}', f'{p}B_{ad}')

    xd = (x * m_d[..., None]).reshape(TOK, H).T
    xv = (x * m_v[..., None]).reshape(TOK, H).T
    xd = np.ascontiguousarray(xd).astype(ml_dtypes.bfloat16)
    xv = np.ascontiguousarray(xv).astype(ml_dtypes.bfloat16)

    mdb = np.broadcast_to(m_d.reshape(1, TOK), (128, TOK)).astype(np.float32)
    mvb = np.broadcast_to(m_v.reshape(1, TOK), (128, TOK)).astype(np.float32)
    mdb = np.ascontiguousarray(mdb)
    mvb = np.ascontiguousarray(mvb)

    inv = 1.0 / (10000.0 ** (np.arange(0, HD, 2, dtype=np.float32) / HD))
    fr = np.outer(np.arange(S, dtype=np.float32), inv)      # [S, 64]
    cosf = np.cos(fr).T.astype(np.float32)                  # [64, S]
    sinf = np.sin(fr).T.astype(np.float32)
    cosT = np.ascontiguousarray(np.vstack([cosf, cosf]))
    sinTs = np.ascontiguousarray(np.vstack([-sinf, sinf]))

    kl = np.arange(128)[:, None]
    ql = np.arange(QB)[None, :]
    cmt = np.concatenate(
        [(j * 128 + kl <= ql).astype(np.float32) for j in range(4)], axis=1)
    cmt = np.ascontiguousarray(cmt)

    in_maps = []
    for c in range(NCORES):
        D = slice(c * DPC, (c + 1) * DPC)
        im = {
            'xd': xd, 'xv': xv, 'mdb': mdb, 'mvb': mvb,
            'cosT': cosT, 'sinTs': sinTs, 'cmt': cmt,
            'wq_d': np.ascontiguousarray(Wf[('q', 'd')][D].T).astype(ml_dtypes.bfloat16),
            'wq_v': np.ascontiguousarray(Wf[('q', 'v')][D].T).astype(ml_dtypes.bfloat16),
            'wk_d': np.ascontiguousarray(Wf[('k', 'd')][D].T).astype(ml_dtypes.bfloat16),
            'wk_v': np.ascontiguousarray(Wf[('k', 'v')][D].T).astype(ml_dtypes.bfloat16),
            'wv_d': np.ascontiguousarray(Wf[('v', 'd')][D].T).astype(ml_dtypes.bfloat16),
            'wv_v': np.ascontiguousarray(Wf[('v', 'v')][D].T).astype(ml_dtypes.bfloat16),
            'wo_d': np.ascontiguousarray(Wf[('o', 'd')][:, D].T).astype(ml_dtypes.bfloat16),
            'wo_v': np.ascontiguousarray(Wf[('o', 'v')][:, D].T).astype(ml_dtypes.bfloat16),
        }
        in_maps.append(im)
    return in_maps


def kernel(**inputs):
    if 'nc' not in _CACHE:
        _CACHE['nc'] = _build()
    nc = _CACHE['nc']
    in_maps = _prep(inputs)
    res = bass_utils.run_bass_kernel_spmd(
        nc, in_maps, core_ids=list(range(NCORES)))
    _CACHE['last_results'] = res
    acc = np.zeros((H, TOK), np.float32)
    for c in range(NCORES):
        acc += res.results[c]['outp']
    return np.ascontiguousarray(acc.T.reshape(B, S, H))



# revision 5
# speedup vs baseline: 9.1249x; 9.1249x over previous
"""LocalLoraAttention Trainium2 kernel: 8-core head-sharded, LoRA folded into
weights, wire-optimized for the axon tunnel.

The end-to-end time is dominated by host<->device transfer over the axon
tunnel (~20-40 MB/s), not device compute (~1ms), so v2 minimizes bytes:
 - host masks x into xd/xv [H,TOK] bf16 and sends each core only a 512-token
   slice (4MB/core); the full xd/xv is rebuilt on-device via AllGather.
 - per-token modal masks go up as [8,512] f32 (8KB) and are broadcast to
   [128,QB] on device with a K=1 matmul.
 - RoPE tables go up as half-tables [64,S]; the causal mask as one [128,512]
   tile; both are expanded on device.
 - the o-projection partials are summed across cores with an on-device
   ReduceScatter; each core downloads only its [256,TOK] shard, cast to bf16.

Compute per core (2 heads, 256 out-dims): LoRA folded on host
(W_d = W + 2*B_d@A_d, W_v likewise), so q/k/v are two 16-chunk matmul chains
into one PSUM, RoPE via swapped-half multiply-add, causal attention with
exp (no max subtraction) and ones-matmul denominator, then a full-width
partial o-projection reduced across cores by the ReduceScatter.
"""
import sys
sys.path.insert(0, '/opt/trn_rl_repo')
import numpy as np
import ml_dtypes

import concourse.bass as bass
import concourse.tile as tile
import concourse.mybir as mybir
from concourse import bass_utils

B, S, H, NH, HD, R = 2, 2048, 2048, 16, 128, 128
LORA_SCALE = 2.0
NCORES = 8
DPC = H // NCORES          # 256 out-dims per core (2 heads)
TOK = B * S                # 4096
NB = 256                   # phase A token block
QB = 512                   # attention q block
NCH = H // 128             # 16 contraction chunks
NKT = S // 128             # 16 k-tiles per batch
NQB = S // QB              # 4 q blocks per batch
SLC = TOK // NCORES        # 512-token input slice per core
F32 = mybir.dt.float32
BF16 = mybir.dt.bfloat16
ISQ = float(1.0 / np.sqrt(HD))
RG = [list(range(NCORES))]

_CACHE = {}


def _split_waits(nc, max_waits=1):
    """This walrus build allows only one sync-wait per instruction; split
    extras onto preceding NOPs on the same engine."""
    ctr = 0
    for fn in nc.m.functions:
        for bb in fn.blocks:
            out = []
            for inst in bb.instructions:
                si = getattr(inst, 'sync_info', None)
                waits = list(si.on_wait) if si and si.on_wait else []
                if len(waits) > max_waits:
                    chunks = [waits[i:i + max_waits]
                              for i in range(0, len(waits), max_waits)]
                    for ch in chunks[:-1]:
                        ctr += 1
                        nop = mybir.InstNoOp(
                            name=f"Wsplit-{ctr}", ins=[], outs=[],
                            sync_info=mybir.SyncInfo(on_wait=ch, on_update=[]))
                        nop.engine = inst.engine
                        out.append(nop)
                    si.on_wait = chunks[-1]
                out.append(inst)
            bb.instructions[:] = out


def _build():
    import concourse.tile_utils as tile_utils
    tile_utils.max_sbuf_usage = 204 * 1024

    nc = bass.Bass("TRN2", target_bir_lowering=False, num_devices=NCORES)
    xin = nc.dram_tensor("xin", [2 * H, SLC], BF16, kind="ExternalInput")
    wq_d = nc.dram_tensor("wq_d", [H, DPC], BF16, kind="ExternalInput")
    wq_v = nc.dram_tensor("wq_v", [H, DPC], BF16, kind="ExternalInput")
    wk_d = nc.dram_tensor("wk_d", [H, DPC], BF16, kind="ExternalInput")
    wk_v = nc.dram_tensor("wk_v", [H, DPC], BF16, kind="ExternalInput")
    wv_d = nc.dram_tensor("wv_d", [H, DPC], BF16, kind="ExternalInput")
    wv_v = nc.dram_tensor("wv_v", [H, DPC], BF16, kind="ExternalInput")
    wo_d = nc.dram_tensor("wo_d", [DPC, H], BF16, kind="ExternalInput")
    wo_v = nc.dram_tensor("wo_v", [DPC, H], BF16, kind="ExternalInput")
    mdr = nc.dram_tensor("mdr", [NCORES, SLC], F32, kind="ExternalInput")
    mvr = nc.dram_tensor("mvr", [NCORES, SLC], F32, kind="ExternalInput")
    cosH = nc.dram_tensor("cosH", [64, S], F32, kind="ExternalInput")
    sinH = nc.dram_tensor("sinH", [64, S], F32, kind="ExternalInput")
    cmt0 = nc.dram_tensor("cmt0", [128, QB], F32, kind="ExternalInput")
    outs = nc.dram_tensor("outs", [DPC, TOK], BF16, kind="ExternalOutput")

    with tile.TileContext(nc) as tc:
        with tc.tile_pool(name="dram", bufs=1, space="DRAM") as dramp, \
             tc.tile_pool(name="wp", bufs=1) as wp, \
             tc.tile_pool(name="qkv", bufs=1) as qkvp, \
             tc.tile_pool(name="xs", bufs=2) as xs, \
             tc.tile_pool(name="rw", bufs=3) as rw, \
             tc.tile_pool(name="ew", bufs=1) as ew, \
             tc.tile_pool(name="at", bufs=2) as atp, \
             tc.tile_pool(name="ad", bufs=2) as adp, \
             tc.tile_pool(name="osp", bufs=2) as osp, \
             tc.tile_pool(name="cst", bufs=2) as cst, \
             tc.tile_pool(name="ps", bufs=8, space="PSUM") as psp:

            # ---- AllGather the xd/xv slices into full [2H, TOK] ----
            ib = dramp.tile([2 * H, SLC], BF16)
            nc.sync.dma_start(out=ib, in_=xin[:, :])
            xg = dramp.tile([NCORES * 2 * H, SLC], BF16, addr_space="Shared")
            nc.gpsimd.collective_compute(
                "AllGather", mybir.AluOpType.bypass, replica_groups=RG,
                ins=[ib.opt()], outs=[xg.opt()])
            # row index a in xg3: a = c0*32 + (0 for xd | 16 for xv) + chunk
            xg3 = xg.rearrange("(a p) t -> p a t", p=128)

            # o-projection partials / ReduceScatter buffers
            outp_loc = dramp.tile([H, TOK], F32)
            rs_out = dramp.tile([DPC, TOK], F32)

            def w3d(dram):  # [H, DPC] -> sbuf [128, NCH, DPC]
                t = wp.tile([128, NCH, DPC], BF16, tag=dram.name)
                nc.sync.dma_start(
                    out=t, in_=dram.rearrange("(c p) d -> p c d", p=128))
                return t

            wq = {'d': w3d(wq_d), 'v': w3d(wq_v)}
            wk = {'d': w3d(wk_d), 'v': w3d(wk_v)}
            wv = {'d': w3d(wv_d), 'v': w3d(wv_v)}
            wo = {}
            for nm, dram in (('d', wo_d), ('v', wo_v)):
                t = wp.tile([128, 2, H], BF16, tag='wo' + nm)
                nc.sync.dma_start(
                    out=t, in_=dram.rearrange("(c p) o -> p c o", p=128))
                wo[nm] = t

            # RoPE tables from half-tables; fold rotate-half sign into sin
            cos_sb = wp.tile([128, S], F32, tag='cos')
            nc.sync.dma_start(out=cos_sb[0:64, :], in_=cosH[:, :])
            nc.sync.dma_start(out=cos_sb[64:128, :], in_=cosH[:, :])
            sin_sb = wp.tile([128, S], F32, tag='sin')
            nc.sync.dma_start(out=sin_sb[0:64, :], in_=sinH[:, :])
            nc.sync.dma_start(out=sin_sb[64:128, :], in_=sinH[:, :])
            nc.scalar.activation(
                sin_sb[0:64, :], sin_sb[0:64, :],
                mybir.ActivationFunctionType.Copy, scale=-1.0)

            # causal mask [128, 4, QB] from the single j=0 tile
            cm0_sb = wp.tile([128, QB], F32, tag='cm0')
            nc.sync.dma_start(out=cm0_sb, in_=cmt0[:, :])
            cm_sb = wp.tile([128, 4, QB], F32, tag='cm')
            nc.vector.memset(cm_sb, 0.0)
            for j in range(4):
                nc.vector.tensor_copy(
                    cm_sb[:, j, j * 128:QB], cm0_sb[:, 0:QB - j * 128])

            ones128 = wp.tile([128, 1], BF16, tag='o128')
            nc.vector.memset(ones128, 1.0)
            ones1 = wp.tile([1, 128], F32, tag='o1')
            nc.vector.memset(ones1, 1.0)

            qT = qkvp.tile([128, 2, S], BF16, tag='qT')
            kT = qkvp.tile([128, 2, S], BF16, tag='kT')
            v_sb = qkvp.tile([128, NKT, 256], BF16, tag='v')

            for b in range(B):
                # ---- phase A: qkv projections for batch b ----
                for t in range(S // NB):
                    tokg = b * S + t * NB
                    s0 = t * NB
                    a0 = (tokg // SLC) * 32
                    o = tokg % SLC
                    xdt = xs.tile([128, NCH, NB], BF16, tag='xd')
                    nc.sync.dma_start(out=xdt, in_=xg3[:, a0:a0 + 16, o:o + NB])
                    xvt = xs.tile([128, NCH, NB], BF16, tag='xv')
                    nc.sync.dma_start(
                        out=xvt, in_=xg3[:, a0 + 16:a0 + 32, o:o + NB])

                    for wdict, dstT in ((wq, qT), (wk, kT)):
                        for hb in range(2):
                            ps = psp.tile([128, NB], F32, tag='ps')
                            i = 0
                            for var, xt in (('d', xdt), ('v', xvt)):
                                for c in range(NCH):
                                    nc.tensor.matmul(
                                        ps,
                                        lhsT=wdict[var][:, c, hb * 128:(hb + 1) * 128],
                                        rhs=xt[:, c, :],
                                        start=(i == 0), stop=(i == 31))
                                    i += 1
                            # RoPE + cast eviction
                            scp = rw.tile([128, NB], F32, tag='scp')
                            nc.vector.tensor_copy(scp, ps)
                            sh = rw.tile([128, NB], F32, tag='sh')
                            nc.sync.dma_start(out=sh[0:64, :], in_=scp[64:128, :])
                            nc.sync.dma_start(out=sh[64:128, :], in_=scp[0:64, :])
                            r1 = rw.tile([128, NB], F32, tag='r1')
                            nc.vector.tensor_mul(r1, ps, cos_sb[:, s0:s0 + NB])
                            r2 = rw.tile([128, NB], F32, tag='r2')
                            nc.vector.tensor_mul(r2, sh, sin_sb[:, s0:s0 + NB])
                            nc.vector.tensor_add(
                                dstT[:, hb, s0:s0 + NB], r1, r2)
                    for tt2 in range(NB // 128):
                        psv = psp.tile([128, 256], F32, tag='ps')
                        i = 0
                        for var, xt in (('d', xdt), ('v', xvt)):
                            for c in range(NCH):
                                nc.tensor.matmul(
                                    psv,
                                    lhsT=xt[:, c, tt2 * 128:(tt2 + 1) * 128],
                                    rhs=wv[var][:, c, :],
                                    start=(i == 0), stop=(i == 31))
                                i += 1
                        nc.vector.tensor_copy(
                            v_sb[:, s0 // 128 + tt2, :], psv)

                # ---- phase B+C per q-block ----
                for qb in range(NQB):
                    q0g = b * S + qb * QB
                    qs = qb * QB
                    mrow = q0g // SLC
                    # [1,512] modal-mask rows for these query tokens
                    # (DVE operands must start at partition 0, so DMA the
                    # row out of DRAM rather than slicing a [8,512] tile)
                    md_row = ew.tile([1, SLC], F32, tag='mdrow')
                    nc.sync.dma_start(out=md_row, in_=mdr[mrow:mrow + 1, :])
                    mv_row = ew.tile([1, SLC], F32, tag='mvrow')
                    nc.sync.dma_start(out=mv_row, in_=mvr[mrow:mrow + 1, :])
                    attn = {}
                    for h in range(2):
                        ps_av = psp.tile([128, QB], F32, tag='ps')
                        ps_den = psp.tile([1, QB], F32, tag='ps')
                        nk = 4 * qb + 4
                        for ki in range(nk):
                            ps_s = psp.tile([128, QB], F32, tag='ps')
                            nc.tensor.matmul(
                                ps_s,
                                lhsT=kT[:, h, ki * 128:(ki + 1) * 128],
                                rhs=qT[:, h, qs:qs + QB],
                                start=True, stop=True)
                            at = atp.tile([128, QB], BF16, tag='at')
                            j = ki - 4 * qb
                            if j >= 0:
                                e32 = ew.tile([128, QB], F32, tag='e32')
                                nc.scalar.activation(
                                    e32, ps_s,
                                    mybir.ActivationFunctionType.Exp, scale=ISQ)
                                nc.vector.tensor_mul(at, e32, cm_sb[:, j, :])
                            else:
                                nc.scalar.activation(
                                    at, ps_s,
                                    mybir.ActivationFunctionType.Exp, scale=ISQ)
                            nc.tensor.matmul(
                                ps_av,
                                lhsT=v_sb[:, ki, h * 128:(h + 1) * 128],
                                rhs=at, start=(ki == 0), stop=(ki == nk - 1))
                            nc.tensor.matmul(
                                ps_den, lhsT=ones128, rhs=at,
                                start=(ki == 0), stop=(ki == nk - 1))
                        rden = ew.tile([1, QB], F32, tag='rden')
                        nc.vector.reciprocal(rden, ps_den)
                        # fold modal masks into the 1/denominator row, then
                        # broadcast each across partitions with a K=1 matmul
                        rdm_d = ew.tile([1, QB], F32, tag='rdmd')
                        nc.vector.tensor_mul(rdm_d, rden, md_row)
                        rdm_v = ew.tile([1, QB], F32, tag='rdmv')
                        nc.vector.tensor_mul(rdm_v, rden, mv_row)
                        ps_bd = psp.tile([128, QB], F32, tag='ps')
                        nc.tensor.matmul(ps_bd, lhsT=ones1, rhs=rdm_d,
                                         start=True, stop=True)
                        rbd = ew.tile([128, QB], F32, tag='rbd')
                        nc.vector.tensor_copy(rbd, ps_bd)
                        ps_bv = psp.tile([128, QB], F32, tag='ps')
                        nc.tensor.matmul(ps_bv, lhsT=ones1, rhs=rdm_v,
                                         start=True, stop=True)
                        rbv = ew.tile([128, QB], F32, tag='rbv')
                        nc.vector.tensor_copy(rbv, ps_bv)
                        ad = adp.tile([128, QB], BF16, tag=f'ad{h}')
                        nc.vector.tensor_mul(ad, ps_av, rbd)
                        av = adp.tile([128, QB], BF16, tag=f'av{h}')
                        nc.vector.tensor_mul(av, ps_av, rbv)
                        attn[(h, 'd')] = ad
                        attn[(h, 'v')] = av
                    # phase C: partial o-projection for these 512 tokens
                    for ob in range(NCH):
                        ps_o = psp.tile([128, QB], F32, tag='ps')
                        i = 0
                        for var in ('d', 'v'):
                            for hl in range(2):
                                nc.tensor.matmul(
                                    ps_o,
                                    lhsT=wo[var][:, hl, ob * 128:(ob + 1) * 128],
                                    rhs=attn[(hl, var)],
                                    start=(i == 0), stop=(i == 3))
                                i += 1
                        osb = osp.tile([128, QB], F32, tag='osb')
                        nc.vector.tensor_copy(osb, ps_o)
                        nc.sync.dma_start(
                            out=outp_loc[ob * 128:(ob + 1) * 128, q0g:q0g + QB],
                            in_=osb)

            # ---- ReduceScatter partials; cast own shard to bf16 ----
            nc.gpsimd.collective_compute(
                "ReduceScatter", mybir.AluOpType.add, replica_groups=RG,
                ins=[outp_loc.opt()], outs=[rs_out.opt()])
            for rh in range(2):
                for ch in range(4):
                    c0 = ch * 1024
                    s1 = cst.tile([128, 1024], F32, tag='s1')
                    nc.sync.dma_start(
                        out=s1,
                        in_=rs_out[rh * 128:(rh + 1) * 128, c0:c0 + 1024])
                    s2 = cst.tile([128, 1024], BF16, tag='s2')
                    nc.vector.tensor_copy(s2, s1)
                    nc.sync.dma_start(
                        out=outs[rh * 128:(rh + 1) * 128, c0:c0 + 1024],
                        in_=s2)
    _split_waits(nc)
    return nc


def _prep(inputs):
    x = np.asarray(inputs['hidden_states'], np.float32)
    m_d = np.asarray(inputs['mask_default'], np.float32)
    m_v = np.asarray(inputs['mask_vision'], np.float32)

    def fold(Wn, An, Bn):
        W = np.asarray(inputs[Wn], np.float32)
        A = np.asarray(inputs[An], np.float32)
        Bm = np.asarray(inputs[Bn], np.float32)
        return (W + LORA_SCALE * (Bm @ A)).astype(np.float32)

    Wf = {}
    for p in 'qkvo':
        for ad in 'dv':
            Wf[(p, ad)] = fold(f'W{p}', f'{p}A_{ad}', f'{p}B_{ad}')

    xd = (x * m_d[..., None]).reshape(TOK, H).T.astype(ml_dtypes.bfloat16)
    xv = (x * m_v[..., None]).reshape(TOK, H).T.astype(ml_dtypes.bfloat16)

    mdr = np.ascontiguousarray(m_d.reshape(NCORES, SLC))
    mvr = np.ascontiguousarray(m_v.reshape(NCORES, SLC))

    inv = 1.0 / (10000.0 ** (np.arange(0, HD, 2, dtype=np.float32) / HD))
    fr = np.outer(np.arange(S, dtype=np.float32), inv)      # [S, 64]
    cosH = np.ascontiguousarray(np.cos(fr).T.astype(np.float32))  # [64, S]
    sinH = np.ascontiguousarray(np.sin(fr).T.astype(np.float32))

    kl = np.arange(128)[:, None]
    ql = np.arange(QB)[None, :]
    cmt0 = np.ascontiguousarray((kl <= ql).astype(np.float32))

    in_maps = []
    for c in range(NCORES):
        D = slice(c * DPC, (c + 1) * DPC)
        T = slice(c * SLC, (c + 1) * SLC)
        im = {
            'xin': np.ascontiguousarray(np.vstack([xd[:, T], xv[:, T]])),
            'mdr': mdr, 'mvr': mvr,
            'cosH': cosH, 'sinH': sinH, 'cmt0': cmt0,
            'wq_d': np.ascontiguousarray(Wf[('q', 'd')][D].T).astype(ml_dtypes.bfloat16),
            'wq_v': np.ascontiguousarray(Wf[('q', 'v')][D].T).astype(ml_dtypes.bfloat16),
            'wk_d': np.ascontiguousarray(Wf[('k', 'd')][D].T).astype(ml_dtypes.bfloat16),
            'wk_v': np.ascontiguousarray(Wf[('k', 'v')][D].T).astype(ml_dtypes.bfloat16),
            'wv_d': np.ascontiguousarray(Wf[('v', 'd')][D].T).astype(ml_dtypes.bfloat16),
            'wv_v': np.ascontiguousarray(Wf[('v', 'v')][D].T).astype(ml_dtypes.bfloat16),
            'wo_d': np.ascontiguousarray(Wf[('o', 'd')][:, D].T).astype(ml_dtypes.bfloat16),
            'wo_v': np.ascontiguousarray(Wf[('o', 'v')][:, D].T).astype(ml_dtypes.bfloat16),
        }
        in_maps.append(im)
    return in_maps


def kernel(**inputs):
    if 'nc' not in _CACHE:
        _CACHE['nc'] = _build()
    nc = _CACHE['nc']
    in_maps = _prep(inputs)
    res = bass_utils.run_bass_kernel_spmd(
        nc, in_maps, core_ids=list(range(NCORES)))
    _CACHE['last_results'] = res
    full = np.concatenate(
        [res.results[c]['outs'] for c in range(NCORES)], axis=0)
    return np.ascontiguousarray(full.astype(np.float32).T.reshape(B, S, H))
